# revision 6
# baseline (speedup 1.0000x reference)
"""AdaptiveGridMerger Trainium2 kernel.

Math: the reference scatters x[b,c,:] into a flat 8x8 grid with bilinear
(4-corner) weights from positions[b,c,:], then matmuls grid_weights
GW [270,64]. The scatter matrix S_b [64,306] (column c = the bilinear
hat weights of channel c) is tiny and depends only on positions, so it
is built on the HOST. The tail output rows 256:270 are folded into it:
  st78[c, 0:64]  = S_b[:, c]
  st78[c, 64:78] = (S_b.T @ GW[256:270].T)[c]   (Wtail fold)
so mm1 (lhsT=st78) produces gv[0:64] = S@x AND gv[64:78] = out[256:270]
in one pass. mm2 (lhsT=GW[0:256].T) produces out[0:256] from gv[0:64].

Device work: 6 contiguous [128,*] read DMAs on the sync HWDGE ring
(strict FIFO = reads get full HBM rate before any write), one small
st+gw read on the otherwise-idle scalar HWDGE ring (its tiny
latency-bound descriptors would head-of-line block the x stream),
bf16 matmuls, PSUM->SBUF cast copies alternating DVE/ACT, write DMAs
on the sync ring in readiness order.

Per batch the contraction is accumulated in arrival order c1 -> c0 ->
c2 (c2, packed 50+50 rows x 2048 cols into one [128,2048] read, is the
smallest, last-arriving chunk), so the long reads stream while the PE
works and only 8 small stop-matmuls + the evac pipeline hang off the
final read. Sharding: data-parallel over batch, 2 batches per core.
PSUM: one pool of 4 x [128,1024] f32 slots = exactly 8 banks; spin
matmuls pre-ramp the PE clock during the DMA lead-in.
"""

import numpy as np

import concourse.bass as bass
import concourse.bacc as bacc
import concourse.mybir as mybir
from concourse import tile
from concourse.bass_utils import run_bass_kernel_spmd

B, C, T = 16, 306, 4096
M, G, GS = 270, 64, 8
N_CORES = 8
BL = B // N_CORES  # batches per core

W78 = G + 14          # st block width: 64 grid cols + 14 folded tail cols
XC = T // 2
SB = 4 * W78          # st cols per batch (312)
GC = BL * SB          # gw base col in stgw (624)
WSG = GC + 256        # stgw width (880)
T_PS = 512
N_SPIN = 8

MM_DTYPE = mybir.dt.bfloat16
NP_MM = mybir.dt.np(MM_DTYPE)
FP32 = mybir.dt.float32


def build_nc():
    nc = bacc.Bacc()
    stgw_ext = nc.declare_dram_parameter("stgw", [128, WSG], MM_DTYPE, isOutput=False)
    x2_ext = nc.declare_dram_parameter("x2", [BL, 128, XC], MM_DTYPE, isOutput=False)
    x01_ext = nc.declare_dram_parameter("x01", [BL, 2, 128, T], MM_DTYPE, isOutput=False)
    out_ext = nc.declare_dram_parameter("out", [BL, M, T], MM_DTYPE, isOutput=True)

    with tile.TileContext(nc) as tc:
        with (
            tc.tile_pool(name="const", bufs=1) as constp,
            tc.tile_pool(name="xp", bufs=1) as xp,
            tc.tile_pool(name="gvt", bufs=2) as gvtp,
            tc.tile_pool(name="op", bufs=2) as outp,
            tc.tile_pool(name="ps", bufs=4, space=bass.MemorySpace.PSUM) as psp,
        ):
            # st + gw ride the scalar HWDGE ring: tiny latency-bound
            # descriptors that must not head-of-line block the x stream.
            stgw = constp.tile([128, WSG], MM_DTYPE, tag="stgw")
            nc.scalar.dma_start(out=stgw[:], in_=stgw_ext[:])

            # PE clock pre-ramp while the first reads stream in.
            dummy = constp.tile([128, T_PS], MM_DTYPE, tag="dummy")
            nc.vector.memset(dummy[:], 0.0)
            spin_ps = psp.tile([128, 2 * T_PS], FP32, tag="pb", name="spin_ps")
            for _ in range(N_SPIN):
                nc.tensor.matmul(
                    spin_ps[:, :T_PS], dummy[:, :128], dummy[:], start=True, stop=True
                )

            # ---- x reads on the sync ring, in consumption order
            x2 = {}
            xc0 = {}
            xc1 = {}
            for b in range(BL):
                xc1[b] = xp.tile([128, T], MM_DTYPE, tag=f"xc1_{b}", name=f"xc1_{b}")
                nc.sync.dma_start(out=xc1[b][:], in_=x01_ext[b, 1])
                xc0[b] = xp.tile([128, T], MM_DTYPE, tag=f"xc0_{b}", name=f"xc0_{b}")
                nc.sync.dma_start(out=xc0[b][:], in_=x01_ext[b, 0])
                x2[b] = xp.tile([128, XC], MM_DTYPE, tag=f"x2_{b}", name=f"x2_{b}")
                nc.sync.dma_start(out=x2[b][:], in_=x2_ext[b])

            k_state = {"k": 0}

            def evac(dst, src):
                if k_state["k"] % 2 == 0:
                    nc.vector.tensor_copy(dst, src)
                else:
                    nc.scalar.copy(dst, src)
                k_state["k"] += 1

            out_sb = {}
            for b in range(BL):
                for mi in range(2):
                    out_sb[(b, mi)] = outp.tile(
                        [128, T], MM_DTYPE, tag=f"o{mi}", name=f"o{b}_{mi}"
                    )

            for b in range(BL):
                gvt = gvtp.tile([W78, T], MM_DTYPE, tag="gvt", name=f"gvt{b}")
                gv = {}
                for w in range(4):
                    gv[w] = psp.tile(
                        [128, 2 * T_PS], FP32, tag="pb", name=f"gv{b}_{w}"
                    )

                def mm1(w, which, start, stop):
                    # wave w covers T cols [w*1024, (w+1)*1024); tt = w // 2
                    if which == 2:  # packed c2: tt0 rows 0:50, tt1 rows 64:114
                        p0 = 0 if w < 2 else 64
                        blk = 2 if w < 2 else 3
                        lhs = stgw[p0 : p0 + 50, b * SB + blk * W78 : b * SB + (blk + 1) * W78]
                        rhs_t, rbase, rp0, rn = x2[b], (w % 2) * 2 * T_PS, p0, 50
                    else:
                        lhs = stgw[0:128, b * SB + which * W78 : b * SB + (which + 1) * W78]
                        rhs_t = xc0[b] if which == 0 else xc1[b]
                        rbase, rp0, rn = w * 2 * T_PS, 0, 128
                    for q in range(2):
                        f0 = rbase + q * T_PS
                        nc.tensor.matmul(
                            gv[w][:W78, q * T_PS : (q + 1) * T_PS],
                            lhs,
                            rhs_t[rp0 : rp0 + rn, f0 : f0 + T_PS],
                            start=start,
                            stop=stop,
                            skip_group_check=True,
                        )

                # accumulate in arrival order: c1 (start) -> c0 -> c2 (stop)
                for w in range(4):
                    mm1(w, 1, True, False)
                for w in range(4):
                    mm1(w, 0, False, False)
                for w in range(4):
                    mm1(w, 2, False, True)
                    evac(
                        gvt[:W78, w * 2 * T_PS : (w + 1) * 2 * T_PS],
                        gv[w][:W78],
                    )

                if b == 0:
                    # tail rows 64:78 of gvt are final output rows 256:270
                    nc.sync.dma_start(out=out_ext[b, 256:M, :], in_=gvt[G:W78, :])

                # mm2 per wave; write each [128,2048] half as it completes
                for w in range(4):
                    for mi in range(2):
                        o_ps = psp.tile([128, 2 * T_PS], FP32, tag="pb", name="o_ps")
                        c0 = w * 2 * T_PS
                        for q in range(2):
                            nc.tensor.matmul(
                                o_ps[:, q * T_PS : (q + 1) * T_PS],
                                stgw[0:G, GC + mi * 128 : GC + (mi + 1) * 128],
                                gvt[:G, c0 + q * T_PS : c0 + (q + 1) * T_PS],
                                start=True,
                                stop=True,
                                skip_group_check=True,
                            )
                        evac(out_sb[(b, mi)][:, c0 : c0 + 2 * T_PS], o_ps[:])
                    if w % 2 == 1:
                        tt = w // 2
                        for mi in range(2):
                            nc.sync.dma_start(
                                out=out_ext[b, mi * 128 : (mi + 1) * 128, tt * XC : (tt + 1) * XC],
                                in_=out_sb[(b, mi)][:, tt * XC : (tt + 1) * XC],
                            )
                if b == 1:
                    nc.sync.dma_start(out=out_ext[b, 256:M, :], in_=gvt[G:W78, :])
    nc.compile()
    return nc


def _host_st(positions, grid_weights):
    """st78 [B, C, 78] f32: bilinear hat weights + folded tail rows."""
    gp = (positions.astype(np.float32) + 1.0) * (GS / 2.0)  # [B, C, 2]
    i = np.arange(GS, dtype=np.float32)
    wy = np.maximum(0.0, 1.0 - np.abs(i[None, None, :] - gp[:, :, 0:1]))
    wx = np.maximum(0.0, 1.0 - np.abs(i[None, None, :] - gp[:, :, 1:2]))
    s = (wy[:, :, :, None] * wx[:, :, None, :]).reshape(B, C, G)
    wtail = s @ grid_weights[256:M].T.astype(np.float32)  # [B, C, 14]
    return np.concatenate([s, wtail], axis=2)


def make_in_maps(x, positions, grid_weights):
    st78 = _host_st(positions, grid_weights)
    gw = np.ascontiguousarray(grid_weights[:256].T).astype(NP_MM)  # [64, 256]
    x_mm = x.astype(NP_MM)
    in_maps = []
    for i in range(N_CORES):
        sl = slice(i * BL, (i + 1) * BL)
        stgw_pack = np.zeros((128, WSG), dtype=np.float32)
        x2_pack = np.zeros((BL, 128, XC), dtype=NP_MM)
        for b2 in range(BL):
            gb = i * BL + b2
            o = b2 * SB
            stgw_pack[:, o : o + W78] = st78[gb, 0:128]
            stgw_pack[:, o + W78 : o + 2 * W78] = st78[gb, 128:256]
            stgw_pack[0:50, o + 2 * W78 : o + 3 * W78] = st78[gb, 256:C]
            stgw_pack[64:114, o + 3 * W78 : o + 4 * W78] = st78[gb, 256:C]
            xc2 = x_mm[gb, 256:C].reshape(50, 2, XC)
            x2_pack[b2, 0:50] = xc2[:, 0]
            x2_pack[b2, 64:114] = xc2[:, 1]
        stgw_pack[0:64, GC : GC + 256] = gw
        in_maps.append(
            {
                "stgw": stgw_pack.astype(NP_MM),
                "x2": x2_pack,
                "x01": np.ascontiguousarray(x_mm[sl, 0:256]).reshape(BL, 2, 128, T),
            }
        )
    return in_maps


_NC_CACHE = None


def kernel(x, positions, grid_weights):
    global _NC_CACHE
    if _NC_CACHE is None:
        _NC_CACHE = build_nc()
    nc = _NC_CACHE
    in_maps = make_in_maps(x, positions, grid_weights)
    res = run_bass_kernel_spmd(nc, in_maps, core_ids=list(range(N_CORES)))
    out = np.concatenate([r["out"] for r in res.results], axis=0)
    return np.asarray(out, dtype=np.float32)


if __name__ == "__main__":
    xs = np.random.randn(B, C, T).astype(np.float32)
    ps = np.random.uniform(-1, 0.74, (B, C, 2)).astype(np.float32)
    gw = np.random.randn(M, G).astype(np.float32)
    out = kernel(xs, ps, gw)
    print(out.shape, out.dtype)


# revision 7
# speedup vs baseline: 1.2347x; 1.2347x over previous
"""AdaptiveGridMerger Trainium2 kernel.

Math: the reference scatters x[b,c,:] into a flat 8x8 grid with bilinear
(4-corner) weights from positions[b,c,:], then matmuls grid_weights
GW [270,64]. The scatter matrix S_b [64,306] (column c = the bilinear
hat weights of channel c) is tiny and depends only on positions, so it
is built on the HOST. The tail output rows 256:270 are folded into it:
  st78[c, 0:64]  = S_b[:, c]
  st78[c, 64:78] = (S_b.T @ GW[256:270].T)[c]   (Wtail fold)
so mm1 (lhsT=st78) produces gv[0:64] = S@x AND gv[64:78] = out[256:270]
in one pass. mm2 (lhsT=GW[0:256].T) produces out[0:256] from gv[0:64].

Device work: 6 contiguous [128,*] read DMAs on the sync HWDGE ring
(strict FIFO = reads get full HBM rate before any write), one small
st+gw read on the otherwise-idle scalar HWDGE ring (its tiny
latency-bound descriptors would head-of-line block the x stream),
bf16 matmuls, PSUM->SBUF cast copies alternating DVE/ACT, write DMAs
on the sync ring in readiness order.

Per batch the contraction is accumulated in arrival order c1 -> c0 ->
c2 (c2, packed 50+50 rows x 2048 cols into one [128,2048] read, is the
smallest, last-arriving chunk), so the long reads stream while the PE
works and only 8 small stop-matmuls + the evac pipeline hang off the
final read. Sharding: data-parallel over batch, 2 batches per core.
PSUM: one pool of 4 x [128,1024] f32 slots = exactly 8 banks; spin
matmuls pre-ramp the PE clock during the DMA lead-in.
"""

import numpy as np

import concourse.bass as bass
import concourse.bacc as bacc
import concourse.mybir as mybir
from concourse import tile
from concourse.bass_utils import run_bass_kernel_spmd

B, C, T = 16, 306, 4096
M, G, GS = 270, 64, 8
N_CORES = 8
BL = B // N_CORES  # batches per core

W78 = G + 14          # st block width: 64 grid cols + 14 folded tail cols
XC = T // 2
SB = 4 * W78          # st cols per batch (312)
SE0 = T               # st base col inside xc1e
GE0 = T + SB          # gw base col inside xc1e (4408)
WX1 = GE0 + 256       # xc1e width (4664)
T_PS = 512
N_SPIN = 8

MM_DTYPE = mybir.dt.bfloat16
NP_MM = mybir.dt.np(MM_DTYPE)
FP32 = mybir.dt.float32


def build_nc():
    nc = bacc.Bacc()
    x2_ext = nc.declare_dram_parameter("x2", [BL, 128, XC], MM_DTYPE, isOutput=False)
    xc1e_ext = nc.declare_dram_parameter("xc1e", [BL, 128, WX1], MM_DTYPE, isOutput=False)
    xc0_ext = nc.declare_dram_parameter("xc0", [BL, 128, T], MM_DTYPE, isOutput=False)
    out_ext = nc.declare_dram_parameter("out", [BL, M, T], MM_DTYPE, isOutput=True)

    with tile.TileContext(nc) as tc:
        with (
            tc.tile_pool(name="const", bufs=1) as constp,
            tc.tile_pool(name="xp", bufs=1) as xp,
            tc.tile_pool(name="gvt", bufs=2) as gvtp,
            tc.tile_pool(name="op", bufs=2) as outp,
            tc.tile_pool(name="ps", bufs=4, space=bass.MemorySpace.PSUM) as psp,
        ):
            # PE clock pre-ramp while the first reads stream in.
            dummy = constp.tile([128, T_PS], MM_DTYPE, tag="dummy")
            nc.vector.memset(dummy[:], 0.0)
            spin_ps = psp.tile([128, 2 * T_PS], FP32, tag="pb", name="spin_ps")
            for _ in range(N_SPIN):
                nc.tensor.matmul(
                    spin_ps[:, :T_PS], dummy[:, :128], dummy[:], start=True, stop=True
                )

            # ---- x reads on the sync ring, in consumption order
            x2 = {}
            xc0 = {}
            xc1 = {}
            for b in range(BL):
                # xc1e = x rows 128:256 ++ st blocks ++ gw, one contiguous read
                xc1[b] = xp.tile([128, WX1], MM_DTYPE, tag=f"xc1_{b}", name=f"xc1_{b}")
                nc.sync.dma_start(out=xc1[b][:], in_=xc1e_ext[b])
                xc0[b] = xp.tile([128, T], MM_DTYPE, tag=f"xc0_{b}", name=f"xc0_{b}")
                nc.sync.dma_start(out=xc0[b][:], in_=xc0_ext[b])
                x2[b] = xp.tile([128, XC], MM_DTYPE, tag=f"x2_{b}", name=f"x2_{b}")
                nc.sync.dma_start(out=x2[b][:], in_=x2_ext[b])

            k_state = {"k": 0}

            def evac(dst, src):
                if k_state["k"] % 2 == 0:
                    nc.vector.tensor_copy(dst, src)
                else:
                    nc.scalar.copy(dst, src)
                k_state["k"] += 1

            out_sb = {}
            for b in range(BL):
                for mi in range(2):
                    out_sb[(b, mi)] = outp.tile(
                        [128, T], MM_DTYPE, tag=f"o{mi}", name=f"o{b}_{mi}"
                    )

            for b in range(BL):
                gvt = gvtp.tile([W78, T], MM_DTYPE, tag="gvt", name=f"gvt{b}")
                gv = {}
                for w in range(4):
                    gv[w] = psp.tile(
                        [128, 2 * T_PS], FP32, tag="pb", name=f"gv{b}_{w}"
                    )

                def mm1(w, which, start, stop):
                    # wave w covers T cols [w*1024, (w+1)*1024); tt = w // 2
                    if which == 2:  # packed c2: tt0 rows 0:50, tt1 rows 64:114
                        p0 = 0 if w < 2 else 64
                        blk = 2 if w < 2 else 3
                        lhs = xc1[b][p0 : p0 + 50, SE0 + blk * W78 : SE0 + (blk + 1) * W78]
                        rhs_t, rbase, rp0, rn = x2[b], (w % 2) * 2 * T_PS, p0, 50
                    else:
                        lhs = xc1[b][0:128, SE0 + which * W78 : SE0 + (which + 1) * W78]
                        rhs_t = xc0[b] if which == 0 else xc1[b]
                        rbase, rp0, rn = w * 2 * T_PS, 0, 128
                    for q in range(2):
                        f0 = rbase + q * T_PS
                        nc.tensor.matmul(
                            gv[w][:W78, q * T_PS : (q + 1) * T_PS],
                            lhs,
                            rhs_t[rp0 : rp0 + rn, f0 : f0 + T_PS],
                            start=start,
                            stop=stop,
                            skip_group_check=True,
                        )

                # accumulate in arrival order: c1 (start) -> c0 -> c2 (stop)
                for w in range(4):
                    mm1(w, 1, True, False)
                for w in range(4):
                    mm1(w, 0, False, False)
                for w in range(4):
                    mm1(w, 2, False, True)
                    evac(
                        gvt[:W78, w * 2 * T_PS : (w + 1) * 2 * T_PS],
                        gv[w][:W78],
                    )

                if b == 0:
                    # tail rows 64:78 of gvt are final output rows 256:270
                    nc.sync.dma_start(out=out_ext[b, 256:M, :], in_=gvt[G:W78, :])

                # mm2 per wave; write each [128,2048] half as it completes
                for w in range(4):
                    for mi in range(2):
                        o_ps = psp.tile([128, 2 * T_PS], FP32, tag="pb", name="o_ps")
                        c0 = w * 2 * T_PS
                        for q in range(2):
                            nc.tensor.matmul(
                                o_ps[:, q * T_PS : (q + 1) * T_PS],
                                xc1[b][0:G, GE0 + mi * 128 : GE0 + (mi + 1) * 128],
                                gvt[:G, c0 + q * T_PS : c0 + (q + 1) * T_PS],
                                start=True,
                                stop=True,
                                skip_group_check=True,
                            )
                        evac(out_sb[(b, mi)][:, c0 : c0 + 2 * T_PS], o_ps[:])
                    if w % 2 == 1:
                        tt = w // 2
                        for mi in range(2):
                            nc.sync.dma_start(
                                out=out_ext[b, mi * 128 : (mi + 1) * 128, tt * XC : (tt + 1) * XC],
                                in_=out_sb[(b, mi)][:, tt * XC : (tt + 1) * XC],
                            )
                if b == 1:
                    nc.sync.dma_start(out=out_ext[b, 256:M, :], in_=gvt[G:W78, :])
    nc.compile()
    return nc


def _host_st(positions, grid_weights):
    """st78 [B, C, 78] f32: bilinear hat weights + folded tail rows."""
    gp = (positions.astype(np.float32) + 1.0) * (GS / 2.0)  # [B, C, 2]
    i = np.arange(GS, dtype=np.float32)
    wy = np.maximum(0.0, 1.0 - np.abs(i[None, None, :] - gp[:, :, 0:1]))
    wx = np.maximum(0.0, 1.0 - np.abs(i[None, None, :] - gp[:, :, 1:2]))
    s = (wy[:, :, :, None] * wx[:, :, None, :]).reshape(B, C, G)
    wtail = s @ grid_weights[256:M].T.astype(np.float32)  # [B, C, 14]
    return np.concatenate([s, wtail], axis=2)


def make_in_maps(x, positions, grid_weights):
    st78 = _host_st(positions, grid_weights)
    gw = np.ascontiguousarray(grid_weights[:256].T).astype(NP_MM)  # [64, 256]
    x_mm = x.astype(NP_MM)
    in_maps = []
    for i in range(N_CORES):
        sl = slice(i * BL, (i + 1) * BL)
        xc1e_pack = np.zeros((BL, 128, WX1), dtype=np.float32)
        x2_pack = np.zeros((BL, 128, XC), dtype=NP_MM)
        for b2 in range(BL):
            gb = i * BL + b2
            xc1e_pack[b2, :, 0:T] = x_mm[gb, 128:256].astype(np.float32)
            xc1e_pack[b2, :, SE0 : SE0 + W78] = st78[gb, 0:128]
            xc1e_pack[b2, :, SE0 + W78 : SE0 + 2 * W78] = st78[gb, 128:256]
            xc1e_pack[b2, 0:50, SE0 + 2 * W78 : SE0 + 3 * W78] = st78[gb, 256:C]
            xc1e_pack[b2, 64:114, SE0 + 3 * W78 : SE0 + 4 * W78] = st78[gb, 256:C]
            xc1e_pack[b2, 0:64, GE0 : GE0 + 256] = gw
            xc2 = x_mm[gb, 256:C].reshape(50, 2, XC)
            x2_pack[b2, 0:50] = xc2[:, 0]
            x2_pack[b2, 64:114] = xc2[:, 1]
        in_maps.append(
            {
                "xc1e": xc1e_pack.astype(NP_MM),
                "x2": x2_pack,
                "xc0": np.ascontiguousarray(x_mm[sl, 0:128]),
            }
        )
    return in_maps


_NC_CACHE = None


def kernel(x, positions, grid_weights):
    global _NC_CACHE
    if _NC_CACHE is None:
        _NC_CACHE = build_nc()
    nc = _NC_CACHE
    in_maps = make_in_maps(x, positions, grid_weights)
    res = run_bass_kernel_spmd(nc, in_maps, core_ids=list(range(N_CORES)))
    out = np.concatenate([r["out"] for r in res.results], axis=0)
    return np.asarray(out, dtype=np.float32)


if __name__ == "__main__":
    xs = np.random.randn(B, C, T).astype(np.float32)
    ps = np.random.uniform(-1, 0.74, (B, C, 2)).astype(np.float32)
    gw = np.random.randn(M, G).astype(np.float32)
    out = kernel(xs, ps, gw)
    print(out.shape, out.dtype)
